# revision 32
# baseline (speedup 1.0000x reference)
"""Trainium2 Bass kernel for BertSimSelfAttention (sparse_attention).

Problem (full): B=4, M=64, SEQ=256, DIM=1024, H=16, HD=64.
Effective batch rows R = B*SEQ = 1024, each row: m=64 tokens of dim=1024.
  hs  = transpose(hidden_states,(0,2,1,3)).reshape(R, 64, 1024)
  q/k/v = hs @ W{q,k,v}.T + b   (per token)
  per (row, head): scores = (q @ k.T)/8 * sim[row] + (-1e4)*(1-am[row,j])
  probs = softmax_j(scores);  ctx = probs @ v  -> out [R, 64, 1024]

Sharding: data-parallel over rows, 128 rows/core x 8 cores. The host
pre-transposes x and W so the device consumes contraction-major layouts
directly (layout prep is part of the shard step).

Per-core design:
  - xT [d, t] and WT [d, o] loaded d-major, rounded to fp32r on DVE.
  - Projections in fp32r (1 cyc/row on PE): qT/kT [o, t] bf16
    (heads on partition strips by parity), v natural [t, o] bf16,
    masked by am and biased at evacuation.
  - scores per (row, head): bf16 paired matmuls via tile_position
    (head-even strips 0, head-odd 64) into one PSUM bank
    [128 = 2x64 q, 512 = 8 head-pairs x 64 j] (fp32).
  - softmax: t = S*sim (DVE, sim repeated via stride-0 AP), += mask
    (DVE; mask bcast built by identity-column matmul), exp (ACT),
    per-block reduce + reciprocal + normalize (DVE) -> probs bf16.
    No max-subtraction needed (|scores| <= ~8); masked lanes hit
    exp(-1e4) == 0 exactly.
  - probs transposed per head-pair ([128, 64] PE transposes) into two
    shared PSUM banks, row-parity selects the partition strip via
    tile_position; two [128, 512] evacuations per row-pair.
  - ctx: bf16 matmuls pairing (row_even, row_odd) per head so the
    natural v layout needs no duplication; PSUM -> SBUF (ACT) -> DRAM.
"""
import sys

sys.path.insert(0, "/opt/trn_rl_repo")

import numpy as np
import concourse.bass as bass
import concourse.bacc as bacc
import concourse.mybir as mybir
import concourse.tile as tile

F32 = mybir.dt.float32
F32R = mybir.dt.float32r
BF16 = mybir.dt.bfloat16
AF = mybir.ActivationFunctionType
ALU = mybir.AluOpType

N_CORES = 8
M = 64                    # tokens per row
DIM = 1024
H = 16
HD = 64
NEG = -10000.0


def build_core_kernel(nc, n_tiles=16, rows_per_tile=8, debug=False, use_bv=True):
    """Emit the per-core program. tile = rows_per_tile rows (must be even)."""
    T_TILE = rows_per_tile * M        # tokens per tile (512 default)
    n_rows = n_tiles * rows_per_tile
    n_tok = n_rows * M
    SUB = T_TILE // 128               # 128-token subtiles per tile

    xt_d = nc.dram_tensor("xT", (DIM, n_tok), F32, kind="ExternalInput")
    sim_d = nc.dram_tensor("simg", (n_rows, M, M), F32, kind="ExternalInput")
    am_d = nc.dram_tensor("am", (n_rows, M), F32, kind="ExternalInput")
    wq_d = nc.dram_tensor("WqT", (DIM, DIM), F32, kind="ExternalInput")
    wk_d = nc.dram_tensor("WkT", (DIM, DIM), F32, kind="ExternalInput")
    wv_d = nc.dram_tensor("WvT", (DIM, DIM), F32, kind="ExternalInput")
    bq_d = nc.dram_tensor("bq", (DIM,), F32, kind="ExternalInput")
    bk_d = nc.dram_tensor("bk", (DIM,), F32, kind="ExternalInput")
    bv_d = nc.dram_tensor("bv", (DIM,), F32, kind="ExternalInput")
    id_d = nc.dram_tensor("ident", (128, 128), F32, kind="ExternalInput")
    sel_d = nc.dram_tensor("selm", (128, 2), F32, kind="ExternalInput")
    bsel_d = nc.dram_tensor("bselm", (2, 128), F32, kind="ExternalInput")
    out_d = nc.dram_tensor("out", (n_tok, DIM), F32, kind="ExternalOutput")

    dbg = {}
    if debug:
        dbg["qt"] = nc.dram_tensor("dbg_qt", (DIM, n_tok), F32, kind="ExternalOutput")
        dbg["kt"] = nc.dram_tensor("dbg_kt", (DIM, n_tok), F32, kind="ExternalOutput")
        dbg["v"] = nc.dram_tensor("dbg_v", (n_tok, DIM), F32, kind="ExternalOutput")
        dbg["pr"] = nc.dram_tensor("dbg_pr", (n_rows, 128, 512), F32,
                                   kind="ExternalOutput")
        dbg["s"] = nc.dram_tensor("dbg_s", (n_rows, 128, 512), F32,
                                  kind="ExternalOutput")

    with tile.TileContext(nc) as tc:
        with (
            tc.tile_pool(name="consts", bufs=1) as consts,
            tc.tile_pool(name="stage", bufs=2) as stage,
            tc.tile_pool(name="xtp", bufs=2) as xtp,
            tc.tile_pool(name="qkp", bufs=2) as qkp,
            tc.tile_pool(name="vp", bufs=2) as vp,
            tc.tile_pool(name="rowp", bufs=2) as rowp,
            tc.tile_pool(name="small_ps", bufs=2, space="PSUM") as small_ps,
            tc.tile_pool(name="proj_ps", bufs=2, space="PSUM") as proj_ps,
            tc.tile_pool(name="att_ps", bufs=4, space="PSUM") as att_ps,
        ):
            # ---------------- tiny consts first ----------------
            ident = consts.tile([128, 128], F32)
            nc.sync.dma_start(ident[:], id_d[:])

            am_all = consts.tile([128, M], F32)
            if n_rows < 128:
                nc.gpsimd.memset(am_all[:], 1.0)
            nc.sync.dma_start(am_all[0:n_rows, :], am_d[:])

            # selector weights for partition-direction sums / broadcasts
            sel_f = consts.tile([128, 2], F32)
            nc.sync.dma_start(sel_f[:], sel_d[:])
            sel = consts.tile([128, 2], BF16)
            nc.vector.tensor_copy(sel[:], sel_f[:])
            bsel_f = consts.tile([2, 128], F32)
            nc.sync.dma_start(bsel_f[:], bsel_d[:])
            bsel = consts.tile([2, 128], BF16)
            nc.vector.tensor_copy(bsel[:], bsel_f[:])

            bq_sb = consts.tile([128, 8], F32)
            bk_sb = consts.tile([128, 8], F32)
            nc.sync.dma_start(bq_sb[:], bq_d[:].rearrange("(o p) -> p o", p=128))
            nc.sync.dma_start(bk_sb[:], bk_d[:].rearrange("(o p) -> p o", p=128))

            if use_bv:
                # bv as a K=1 fp32r pair for psum-accumulate
                ones_f = consts.tile([1, 128], F32)
                nc.gpsimd.memset(ones_f[:], 1.0)
                ones_r = consts.tile([1, 128], F32R)
                nc.vector.tensor_copy(ones_r[:], ones_f[:])
                bv_row = consts.tile([1, DIM], F32)
                nc.sync.dma_start(bv_row[:],
                                  bv_d[:].rearrange("(a o) -> a o", a=1))
                bv_r = consts.tile([1, DIM], F32R)
                nc.vector.tensor_copy(bv_r[:], bv_row[:])

            # mask bias columns: mcolT2[:, r] = -1e4*(1 - am[r, j]) on both
            # partition halves (exp-bias per key token j)
            mcolT2 = consts.tile([128, 128], F32)
            amt_ps = small_ps.tile([128, 128], F32, tag="srb")
            nc.tensor.transpose(amt_ps[0:M, 0:128], am_all[:], ident[:])
            nc.vector.tensor_scalar(
                mcolT2[0:64, :], amt_ps[0:M, 0:128], -NEG, NEG,
                op0=ALU.mult, op1=ALU.add)
            nc.vector.tensor_scalar(
                mcolT2[64:128, :], amt_ps[0:M, 0:128], -NEG, NEG,
                op0=ALU.mult, op1=ALU.add)

            # ---------------- weights (+ tile-0 x interleaved) ----------
            def emit_xt(ti):
                t0 = ti * T_TILE
                xt = [xtp.tile([128, T_TILE], F32R, tag=f"xt{d}",
                               name=f"xt{d}_{ti}") for d in range(8)]
                for dch in range(8):
                    xst = stage.tile([128, T_TILE], F32, tag="xstage",
                                     name=f"xst{dch}_{ti}")
                    nc.sync.dma_start(
                        xst[:], xt_d[128 * dch:128 * dch + 128, t0:t0 + T_TILE]
                    )
                    nc.vector.tensor_copy(xt[dch][:], xst[:])
                return xt

            def emit_w(name, w_d, dchs):
                wt = wts[name]
                for dch in dchs:
                    for hh in range(DIM // 512):
                        wnat = stage.tile([128, 512], F32, tag="xstage",
                                          name=f"wn{name}{dch}{hh}")
                        nc.sync.dma_start(
                            wnat[:],
                            w_d[128 * dch:128 * dch + 128,
                                512 * hh:512 * hh + 512])
                        nc.vector.tensor_copy(
                            wt[dch][:, 512 * hh:512 * hh + 512], wnat[:])

            wts = {name: [consts.tile([128, DIM], F32R, tag=f"w{name}{d}",
                                      name=f"w{name}{d}") for d in range(8)]
                   for name in ("q", "k", "v")}
            xt0 = [xtp.tile([128, T_TILE], F32R, tag=f"xt{d}",
                             name=f"xt{d}_0") for d in range(8)]
            for dch in range(8):
                emit_w("q", wq_d, [dch])
                xst = stage.tile([128, T_TILE], F32, tag="xstage",
                                 name=f"xst{dch}_0")
                nc.sync.dma_start(xst[:], xt_d[128 * dch:128 * dch + 128,
                                               0:T_TILE])
                nc.vector.tensor_copy(xt0[dch][:], xst[:])
            emit_w("k", wk_d, range(8))
            emit_w("v", wv_d, range(8))
            wqt, wkt, wvt = wts["q"], wts["k"], wts["v"]

            # ---------------- main loop over token tiles ----------------
            # Emission interleaves tile ti's projection groups with tile
            # (ti-1)'s attention rows so the PE program order has dense
            # matmul work to fill softmax dependency stalls (keeps HAM warm).

            def make_proj(ti, xt):
                qt = [qkp.tile([128, T_TILE], BF16, tag=f"qt{o}",
                               name=f"qt{o}_{ti}") for o in range(8)]
                kt = [qkp.tile([128, T_TILE], BF16, tag=f"kt{o}",
                               name=f"kt{o}_{ti}") for o in range(8)]
                vts = [vp.tile([128, DIM], BF16, tag=f"v{s}",
                               name=f"v{s}_{ti}") for s in range(SUB)]
                groups = []

                def qk_group(wt, dst, b_sb, och):
                    ps = proj_ps.tile([128, T_TILE], F32, tag="proj",
                                      name=f"qkps{och}_{ti}")
                    for dch in range(8):
                        nc.tensor.matmul(
                            ps[:],
                            wt[dch][:, 128 * och:128 * och + 128],
                            xt[dch][:],
                            start=(dch == 0), stop=(dch == 7),
                        )
                    nc.scalar.activation(
                        dst[och][:], ps[:], AF.Identity,
                        bias=b_sb[:, och:och + 1], scale=1.0,
                    )

                def v_group(sub, oh):
                    vt = vts[sub]
                    ps = proj_ps.tile([128, 512], F32, tag="proj",
                                      name=f"vps{sub}{oh}_{ti}")
                    sl = slice(512 * oh, 512 * oh + 512)
                    for dch in range(8):
                        nc.tensor.matmul(
                            ps[:],
                            xt[dch][:, 128 * sub:128 * sub + 128],
                            wvt[dch][:, 512 * oh:512 * oh + 512],
                            start=(dch == 0), stop=(dch == 7) and not use_bv,
                        )
                    if use_bv:
                        nc.tensor.matmul(
                            ps[:], ones_r[:], bv_r[:, sl],
                            start=False, stop=True,
                        )
                    nc.scalar.copy(vt[:, sl], ps[:])

                for wt, dst, b_sb in ((wqt, qt, bq_sb), (wkt, kt, bk_sb)):
                    for och in range(8):
                        groups.append(
                            lambda wt=wt, dst=dst, b_sb=b_sb, och=och:
                            qk_group(wt, dst, b_sb, och))
                for sub in range(SUB):
                    for oh in range(2):
                        groups.append(lambda sub=sub, oh=oh: v_group(sub, oh))
                return qt, kt, vts, groups

            def make_att_rows(ti, qt, kt, vts):
                rowstate = {}

                def att_row_a(rr):
                    r = ti * rows_per_tile + rr

                    simt2 = rowp.tile([128, M], F32, tag="sim2",
                                      name=f"sim2_{r}")
                    nc.sync.dma_start(simt2[0:64, :], sim_d[r, :, :])
                    nc.sync.dma_start(simt2[64:128, :], sim_d[r, :, :])

                    # scores transposed: S'[j, q] (bf16 in, fp32 psum)
                    s_ps = att_ps.tile([128, 512], F32, tag="att",
                                       name=f"s_{r}")
                    tsl = slice(M * rr, M * rr + M)
                    for h in range(H):
                        hp, half = h // 2, h % 2
                        st = 64 * half
                        nc.tensor.matmul(
                            s_ps[st:st + 64, 64 * hp:64 * hp + 64],
                            kt[h // 2][st:st + 64, tsl],
                            qt[h // 2][st:st + 64, tsl],
                            start=True, stop=True,
                            tile_position=(st, st),
                        )

                    # t = S' * simT;  e = exp(t + maskcol)  (bf16 out)
                    tt = rowp.tile([128, 512], F32, tag="tt", name=f"tt_{r}")
                    nc.vector.tensor_tensor(
                        tt[:].rearrange("p (a j) -> p a j", j=M),
                        s_ps[:].rearrange("p (a j) -> p a j", j=M),
                        simt2[:].rearrange("p (a j) -> p a j", a=1)
                        .broadcast_to([128, 8, M]),
                        op=ALU.mult,
                    )
                    et = rowp.tile([128, 512], BF16, tag="et", name=f"et_{r}")
                    nc.scalar.activation(et[:], tt[:], AF.Exp,
                                         bias=mcolT2[:, r:r + 1])

                    # denominators: per-half partition sums via PE
                    dn_ps = small_ps.tile([2, 512], F32, tag="srb",
                                          name=f"dn_{r}")
                    nc.tensor.matmul(dn_ps[:], sel[:], et[:],
                                     start=True, stop=True)
                    rcs = rowp.tile([2, 512], BF16, tag="rcs", name=f"rcs_{r}")
                    with nc.allow_low_precision(
                            reason="bf16 softmax reciprocal, error budget ok"):
                        nc.vector.reciprocal(rcs[:], dn_ps[:])
                    r_ps = small_ps.tile([128, 512], F32, tag="srb",
                                         name=f"rb_{r}")
                    nc.tensor.matmul(r_ps[:], bsel[:], rcs[:],
                                     start=True, stop=True)

                    # probs.T = e * recip (bf16) -- direct ctx stationary
                    pt = rowp.tile([128, 512], BF16, tag="pt", name=f"pt_{r}")
                    nc.vector.tensor_tensor(pt[:], et[:], r_ps[:],
                                            op=ALU.mult)
                    if debug:
                        dpr = stage.tile([128, 512], F32, tag="dbgpr",
                                         name=f"dpr_{r}")
                        nc.scalar.copy(dpr[:], pt[:])
                        nc.gpsimd.dma_start(dbg["pr"][r, :, :], dpr[:])
                        ssb = stage.tile([128, 512], F32, tag="ssb",
                                         name=f"ssb_{r}")
                        nc.scalar.copy(ssb[:], s_ps[:])
                        nc.gpsimd.dma_start(dbg["s"][r, :, :], ssb[:])
                    rowstate[rr] = pt

                def att_row_b(rr):
                    r = ti * rows_per_tile + rr
                    rp = rr % 2
                    pt = rowstate.pop(rr)

                    # v duplicated to both partition strips (SBUF->SBUF DMA)
                    vt = vts[rr // 2]
                    v2 = rowp.tile([128, DIM], BF16, tag="v2", name=f"v2_{r}")
                    nc.sync.dma_start(v2[0:64, :], vt[64 * rp:64 * rp + 64, :])
                    nc.sync.dma_start(v2[64:128, :], vt[64 * rp:64 * rp + 64, :])

                    # ctx: head pairs (even, odd) at partition strips (0, 64)
                    ctx_ps = att_ps.tile([128, 512], F32, tag="att",
                                         name=f"ctx_{r}")
                    for hp in range(8):
                        for par in range(2):
                            h = 2 * hp + par
                            st = 64 * par
                            nc.tensor.matmul(
                                ctx_ps[st:st + 64, 64 * hp:64 * hp + 64],
                                pt[st:st + 64, 64 * hp:64 * hp + 64],
                                v2[st:st + 64, 64 * h:64 * h + 64],
                                start=True, stop=True,
                                tile_position=(st, st),
                            )
                    osb = rowp.tile([128, 512], F32, tag="osb",
                                    name=f"osb_{r}", bufs=1)
                    nc.scalar.copy(osb[:], ctx_ps[:])
                    # out[64r + q, 64h + hd]; strip par holds heads 2hp+par
                    ov = out_d[M * r:M * r + M, :].rearrange(
                        "q (hp two hd) -> q hp two hd", two=2, hd=64)
                    for par in range(2):
                        nc.sync.dma_start(
                            ov[:, :, par, :],
                            osb[64 * par:64 * par + 64, :]
                            .rearrange("q (hp hd) -> q hp hd", hd=64),
                        )

                units = []
                for rr in range(rows_per_tile):
                    units.append(lambda rr=rr: att_row_a(rr))
                    units.append(lambda rr=rr: att_row_b(rr))
                return units

            prev_rows = []
            for ti in range(n_tiles):
                xt = xt0 if ti == 0 else emit_xt(ti)
                qt, kt, vts, groups = make_proj(ti, xt)
                ri = 0
                for gi, g in enumerate(groups):
                    g()
                    while (ri < len(prev_rows)
                           and (gi + 1) * len(prev_rows) // len(groups) > ri):
                        prev_rows[ri]()
                        ri += 1
                while ri < len(prev_rows):
                    prev_rows[ri]()
                    ri += 1
                prev_rows = make_att_rows(ti, qt, kt, vts)
            for row in prev_rows:
                row()

    return dict(out=out_d)


def _prepare_shards(hidden_states, attention_mask, sim_graph, Wq, bq, Wk, bk, Wv, bv,
                    n_cores=N_CORES):
    b, m, seq, dim = hidden_states.shape
    R = b * seq
    hs = np.transpose(np.asarray(hidden_states), (0, 2, 1, 3)).reshape(R, m, dim)
    am = np.ascontiguousarray(
        np.transpose(np.asarray(attention_mask), (0, 2, 1)).reshape(R, m),
        dtype=np.float32)
    sim = np.ascontiguousarray(
        np.transpose(np.asarray(sim_graph), (0, 2, 1)), dtype=np.float32)
    ident = np.eye(128, dtype=np.float32)
    selm = np.zeros((128, 2), np.float32)
    selm[0:64, 0] = 1.0
    selm[64:128, 1] = 1.0
    bselm = np.zeros((2, 128), np.float32)
    bselm[0, 0:64] = 1.0
    bselm[1, 64:128] = 1.0
    WqT = np.ascontiguousarray(np.asarray(Wq).T * 0.125, np.float32)
    WkT = np.ascontiguousarray(np.asarray(Wk).T, np.float32)
    WvT = np.ascontiguousarray(np.asarray(Wv).T, np.float32)
    rows_per_core = R // n_cores
    in_maps = []
    for c in range(n_cores):
        r0 = c * rows_per_core
        xT = np.ascontiguousarray(
            hs[r0:r0 + rows_per_core].reshape(rows_per_core * m, dim).T,
            np.float32)
        in_maps.append(dict(
            xT=xT,
            simg=sim[r0:r0 + rows_per_core],
            am=am[r0:r0 + rows_per_core],
            WqT=WqT, WkT=WkT, WvT=WvT,
            bq=np.ascontiguousarray(np.asarray(bq) * 0.125, np.float32),
            bk=np.ascontiguousarray(bk, np.float32),
            bv=np.ascontiguousarray(bv, np.float32),
            ident=ident, selm=selm, bselm=bselm,
        ))
    return in_maps


_CACHE = {}


def _get_compiled(use_bv=True):
    key = ("nc", use_bv)
    if key not in _CACHE:
        nc = bacc.Bacc("TRN2", target_bir_lowering=False, debug=False)
        build_core_kernel(nc, use_bv=use_bv)
        nc.compile()
        _CACHE[key] = nc
    return _CACHE[key]


LAST_EXEC_NS = [None]


def kernel(hidden_states, attention_mask, sim_graph, Wq, bq, Wk, bk, Wv, bv,
           b=4, m=64, seq=256, dim=1024, **_):
    import os
    from concourse.bass_utils import run_bass_kernel_spmd

    use_bv = bool(np.any(np.asarray(bv)))
    nc = _get_compiled(use_bv=use_bv)
    in_maps = _prepare_shards(hidden_states, attention_mask, sim_graph,
                              Wq, bq, Wk, bk, Wv, bv)
    trace = bool(int(os.environ.get("BERT_TRACE", "0")))
    if trace:
        try:  # register the NTFF hook if the middleware didn't
            from antenv.axon_hooks import (get_axon_ntff_profile_hook,
                                           set_axon_ntff_profile_hook)
            if get_axon_ntff_profile_hook() is None:
                from trn_agent_boot.trn_boot import _ntff_profile_via_ctypes
                set_axon_ntff_profile_hook(
                    _ntff_profile_via_ctypes("/opt/axon/libaxon_pjrt.so"))
        except Exception:
            trace = False
    res = run_bass_kernel_spmd(nc, in_maps, list(range(N_CORES)), trace=trace)
    LAST_EXEC_NS[0] = res.exec_time_ns
    R = int(b) * int(seq)
    out = np.concatenate([res.results[c]["out"] for c in range(N_CORES)], axis=0)
    return out.reshape(R, int(m), int(dim))


# revision 34
# speedup vs baseline: 1.5134x; 1.5134x over previous
"""Trainium2 Bass kernel for BertSimSelfAttention (sparse_attention).

Problem (full): B=4, M=64, SEQ=256, DIM=1024, H=16, HD=64.
Effective batch rows R = B*SEQ = 1024, each row: m=64 tokens of dim=1024.
  hs  = transpose(hidden_states,(0,2,1,3)).reshape(R, 64, 1024)
  q/k/v = hs @ W{q,k,v}.T + b   (per token)
  per (row, head): scores = (q @ k.T)/8 * sim[row] + (-1e4)*(1-am[row,j])
  probs = softmax_j(scores);  ctx = probs @ v  -> out [R, 64, 1024]

Sharding: data-parallel over rows, 128 rows/core x 8 cores. The host
pre-transposes x and W so the device consumes contraction-major layouts
directly (layout prep is part of the shard step).

Per-core design:
  - xT [d, t] and WT [d, o] loaded d-major, rounded to fp32r on DVE.
  - Projections in fp32r (1 cyc/row on PE): qT/kT [o, t] bf16
    (heads on partition strips by parity), v natural [t, o] bf16,
    masked by am and biased at evacuation.
  - scores per (row, head): bf16 paired matmuls via tile_position
    (head-even strips 0, head-odd 64) into one PSUM bank
    [128 = 2x64 q, 512 = 8 head-pairs x 64 j] (fp32).
  - softmax: t = S*sim (DVE, sim repeated via stride-0 AP), += mask
    (DVE; mask bcast built by identity-column matmul), exp (ACT),
    per-block reduce + reciprocal + normalize (DVE) -> probs bf16.
    No max-subtraction needed (|scores| <= ~8); masked lanes hit
    exp(-1e4) == 0 exactly.
  - probs transposed per head-pair ([128, 64] PE transposes) into two
    shared PSUM banks, row-parity selects the partition strip via
    tile_position; two [128, 512] evacuations per row-pair.
  - ctx: bf16 matmuls pairing (row_even, row_odd) per head so the
    natural v layout needs no duplication; PSUM -> SBUF (ACT) -> DRAM.
"""
import sys

sys.path.insert(0, "/opt/trn_rl_repo")

import numpy as np
import concourse.bass as bass
import concourse.bacc as bacc
import concourse.mybir as mybir
import concourse.tile as tile

F32 = mybir.dt.float32
F32R = mybir.dt.float32r
BF16 = mybir.dt.bfloat16
AF = mybir.ActivationFunctionType
ALU = mybir.AluOpType

N_CORES = 8
M = 64                    # tokens per row
DIM = 1024
H = 16
HD = 64
NEG = -10000.0


def build_core_kernel(nc, n_tiles=16, rows_per_tile=8, debug=False, use_bv=True):
    """Emit the per-core program. tile = rows_per_tile rows (must be even)."""
    T_TILE = rows_per_tile * M        # tokens per tile (512 default)
    n_rows = n_tiles * rows_per_tile
    n_tok = n_rows * M
    SUB = T_TILE // 128               # 128-token subtiles per tile

    xt_d = nc.dram_tensor("xT", (DIM, n_tok), F32, kind="ExternalInput")
    sim_d = nc.dram_tensor("simg", (n_rows, M, M), F32, kind="ExternalInput")
    am_d = nc.dram_tensor("am", (n_rows, M), F32, kind="ExternalInput")
    wq_d = nc.dram_tensor("WqT", (DIM, DIM), F32, kind="ExternalInput")
    wk_d = nc.dram_tensor("WkT", (DIM, DIM), F32, kind="ExternalInput")
    wv_d = nc.dram_tensor("WvT", (DIM, DIM), F32, kind="ExternalInput")
    bq_d = nc.dram_tensor("bq", (DIM,), F32, kind="ExternalInput")
    bk_d = nc.dram_tensor("bk", (DIM,), F32, kind="ExternalInput")
    bv_d = nc.dram_tensor("bv", (DIM,), F32, kind="ExternalInput")
    id_d = nc.dram_tensor("ident", (128, 128), F32, kind="ExternalInput")
    sel_d = nc.dram_tensor("selm", (128, 2), F32, kind="ExternalInput")
    bsel_d = nc.dram_tensor("bselm", (2, 128), F32, kind="ExternalInput")
    out_d = nc.dram_tensor("out", (n_tok, DIM), F32, kind="ExternalOutput")

    dbg = {}
    if debug:
        dbg["qt"] = nc.dram_tensor("dbg_qt", (DIM, n_tok), F32, kind="ExternalOutput")
        dbg["kt"] = nc.dram_tensor("dbg_kt", (DIM, n_tok), F32, kind="ExternalOutput")
        dbg["v"] = nc.dram_tensor("dbg_v", (n_tok, DIM), F32, kind="ExternalOutput")
        dbg["pr"] = nc.dram_tensor("dbg_pr", (n_rows, 128, 512), F32,
                                   kind="ExternalOutput")
        dbg["s"] = nc.dram_tensor("dbg_s", (n_rows, 128, 512), F32,
                                  kind="ExternalOutput")

    with tile.TileContext(nc) as tc:
        with (
            tc.tile_pool(name="consts", bufs=1) as consts,
            tc.tile_pool(name="stage", bufs=2) as stage,
            tc.tile_pool(name="xtp", bufs=2) as xtp,
            tc.tile_pool(name="qkp", bufs=2) as qkp,
            tc.tile_pool(name="vp", bufs=2) as vp,
            tc.tile_pool(name="rowp", bufs=2) as rowp,
            tc.tile_pool(name="small_ps", bufs=2, space="PSUM") as small_ps,
            tc.tile_pool(name="proj_ps", bufs=2, space="PSUM") as proj_ps,
            tc.tile_pool(name="att_ps", bufs=4, space="PSUM") as att_ps,
        ):
            # ---------------- tiny consts first ----------------
            ident = consts.tile([128, 128], F32)
            nc.sync.dma_start(ident[:], id_d[:])

            am_all = consts.tile([128, M], F32)
            if n_rows < 128:
                nc.gpsimd.memset(am_all[:], 1.0)
            nc.sync.dma_start(am_all[0:n_rows, :], am_d[:])

            # block-ones selector: halfones[p, m] = 1 iff same 64-half;
            # halfones.T @ e sums each half and broadcasts to its partitions
            halfones = consts.tile([128, 128], BF16)
            nc.gpsimd.memset(halfones[:], 0.0)
            nc.gpsimd.memset(halfones[0:64, 0:64], 1.0)
            nc.gpsimd.memset(halfones[64:128, 64:128], 1.0)

            bq_sb = consts.tile([128, 8], F32)
            bk_sb = consts.tile([128, 8], F32)
            nc.sync.dma_start(bq_sb[:], bq_d[:].rearrange("(o p) -> p o", p=128))
            nc.sync.dma_start(bk_sb[:], bk_d[:].rearrange("(o p) -> p o", p=128))

            if use_bv:
                # bv as a K=1 fp32r pair for psum-accumulate
                ones_f = consts.tile([1, 128], F32)
                nc.gpsimd.memset(ones_f[:], 1.0)
                ones_r = consts.tile([1, 128], F32R)
                nc.vector.tensor_copy(ones_r[:], ones_f[:])
                bv_row = consts.tile([1, DIM], F32)
                nc.sync.dma_start(bv_row[:],
                                  bv_d[:].rearrange("(a o) -> a o", a=1))
                bv_r = consts.tile([1, DIM], F32R)
                nc.vector.tensor_copy(bv_r[:], bv_row[:])

            # mask bias columns: mcolT2[:, r] = -1e4*(1 - am[r, j]) on both
            # partition halves (exp-bias per key token j)
            mcolT2 = consts.tile([128, 128], F32)
            amt_ps = small_ps.tile([128, 128], F32, tag="srb")
            nc.tensor.transpose(amt_ps[0:M, 0:128], am_all[:], ident[:])
            nc.vector.tensor_scalar(
                mcolT2[0:64, :], amt_ps[0:M, 0:128], -NEG, NEG,
                op0=ALU.mult, op1=ALU.add)
            nc.vector.tensor_scalar(
                mcolT2[64:128, :], amt_ps[0:M, 0:128], -NEG, NEG,
                op0=ALU.mult, op1=ALU.add)

            # ---------------- weights (+ tile-0 x interleaved) ----------
            def emit_xt(ti):
                t0 = ti * T_TILE
                xt = [xtp.tile([128, T_TILE], F32R, tag=f"xt{d}",
                               name=f"xt{d}_{ti}") for d in range(8)]
                for dch in range(8):
                    xst = stage.tile([128, T_TILE], F32, tag="xstage",
                                     name=f"xst{dch}_{ti}")
                    nc.sync.dma_start(
                        xst[:], xt_d[128 * dch:128 * dch + 128, t0:t0 + T_TILE]
                    )
                    nc.vector.tensor_copy(xt[dch][:], xst[:])
                return xt

            def emit_w(name, w_d, dchs):
                wt = wts[name]
                for dch in dchs:
                    for hh in range(DIM // 512):
                        wnat = stage.tile([128, 512], F32, tag="xstage",
                                          name=f"wn{name}{dch}{hh}")
                        nc.sync.dma_start(
                            wnat[:],
                            w_d[128 * dch:128 * dch + 128,
                                512 * hh:512 * hh + 512])
                        nc.vector.tensor_copy(
                            wt[dch][:, 512 * hh:512 * hh + 512], wnat[:])

            wts = {name: [consts.tile([128, DIM], F32R, tag=f"w{name}{d}",
                                      name=f"w{name}{d}") for d in range(8)]
                   for name in ("q", "k", "v")}
            xt0 = [xtp.tile([128, T_TILE], F32R, tag=f"xt{d}",
                             name=f"xt{d}_0") for d in range(8)]
            for dch in range(8):
                emit_w("q", wq_d, [dch])
                xst = stage.tile([128, T_TILE], F32, tag="xstage",
                                 name=f"xst{dch}_0")
                nc.sync.dma_start(xst[:], xt_d[128 * dch:128 * dch + 128,
                                               0:T_TILE])
                nc.vector.tensor_copy(xt0[dch][:], xst[:])
            emit_w("k", wk_d, range(8))
            emit_w("v", wv_d, range(8))
            wqt, wkt, wvt = wts["q"], wts["k"], wts["v"]

            # ---------------- main loop over token tiles ----------------
            # Emission interleaves tile ti's projection groups with tile
            # (ti-1)'s attention rows so the PE program order has dense
            # matmul work to fill softmax dependency stalls (keeps HAM warm).

            def make_proj(ti, xt):
                qt = [qkp.tile([128, T_TILE], BF16, tag=f"qt{o}",
                               name=f"qt{o}_{ti}") for o in range(8)]
                kt = [qkp.tile([128, T_TILE], BF16, tag=f"kt{o}",
                               name=f"kt{o}_{ti}") for o in range(8)]
                vts = [vp.tile([128, DIM], BF16, tag=f"v{s}",
                               name=f"v{s}_{ti}") for s in range(SUB)]
                groups = []

                def qk_group(wt, dst, b_sb, och):
                    ps = proj_ps.tile([128, T_TILE], F32, tag="proj",
                                      name=f"qkps{och}_{ti}")
                    for dch in range(8):
                        nc.tensor.matmul(
                            ps[:],
                            wt[dch][:, 128 * och:128 * och + 128],
                            xt[dch][:],
                            start=(dch == 0), stop=(dch == 7),
                        )
                    nc.scalar.activation(
                        dst[och][:], ps[:], AF.Identity,
                        bias=b_sb[:, och:och + 1], scale=1.0,
                    )

                def v_group(sub, oh):
                    vt = vts[sub]
                    ps = proj_ps.tile([128, 512], F32, tag="proj",
                                      name=f"vps{sub}{oh}_{ti}")
                    sl = slice(512 * oh, 512 * oh + 512)
                    for dch in range(8):
                        nc.tensor.matmul(
                            ps[:],
                            xt[dch][:, 128 * sub:128 * sub + 128],
                            wvt[dch][:, 512 * oh:512 * oh + 512],
                            start=(dch == 0), stop=(dch == 7) and not use_bv,
                        )
                    if use_bv:
                        nc.tensor.matmul(
                            ps[:], ones_r[:], bv_r[:, sl],
                            start=False, stop=True,
                        )
                    nc.scalar.copy(vt[:, sl], ps[:])

                for wt, dst, b_sb in ((wqt, qt, bq_sb), (wkt, kt, bk_sb)):
                    for och in range(8):
                        groups.append(
                            lambda wt=wt, dst=dst, b_sb=b_sb, och=och:
                            qk_group(wt, dst, b_sb, och))
                for sub in range(SUB):
                    for oh in range(2):
                        groups.append(lambda sub=sub, oh=oh: v_group(sub, oh))
                return qt, kt, vts, groups

            def make_att_rows(ti, qt, kt, vts):
                rowstate = {}

                def att_row_a(rr):
                    r = ti * rows_per_tile + rr

                    simt2 = rowp.tile([128, M], F32, tag="sim2",
                                      name=f"sim2_{r}")
                    nc.sync.dma_start(simt2[0:64, :], sim_d[r, :, :])
                    nc.sync.dma_start(simt2[64:128, :], sim_d[r, :, :])

                    # scores transposed: S'[j, q] (bf16 in, fp32 psum)
                    s_ps = att_ps.tile([128, 512], F32, tag="att",
                                       name=f"s_{r}")
                    tsl = slice(M * rr, M * rr + M)
                    for h in range(H):
                        hp, half = h // 2, h % 2
                        st = 64 * half
                        nc.tensor.matmul(
                            s_ps[st:st + 64, 64 * hp:64 * hp + 64],
                            kt[h // 2][st:st + 64, tsl],
                            qt[h // 2][st:st + 64, tsl],
                            start=True, stop=True,
                            tile_position=(st, st),
                        )

                    # t = S' * simT;  e = exp(t + maskcol)  (bf16 out)
                    tt = rowp.tile([128, 512], F32, tag="tt", name=f"tt_{r}")
                    nc.vector.tensor_tensor(
                        tt[:].rearrange("p (a j) -> p a j", j=M),
                        s_ps[:].rearrange("p (a j) -> p a j", j=M),
                        simt2[:].rearrange("p (a j) -> p a j", a=1)
                        .broadcast_to([128, 8, M]),
                        op=ALU.mult,
                    )
                    et = rowp.tile([128, 512], BF16, tag="et", name=f"et_{r}")
                    nc.scalar.activation(et[:], tt[:], AF.Exp,
                                         bias=mcolT2[:, r:r + 1])

                    # denominators summed + broadcast in one PE matmul
                    dn_ps = small_ps.tile([128, 512], F32, tag="srb",
                                          name=f"dn_{r}")
                    nc.tensor.matmul(dn_ps[:], halfones[:], et[:],
                                     start=True, stop=True)

                    # in-place approx reciprocal on PSUM (~18 bits), then
                    # probs.T = e * (1/denom) (bf16) -- direct ctx stationary
                    nc.vector.reciprocal_approx_fast(out=dn_ps[:], in_=dn_ps[:])
                    pt = rowp.tile([128, 512], BF16, tag="pt", name=f"pt_{r}")
                    nc.vector.tensor_tensor(pt[:], et[:], dn_ps[:],
                                            op=ALU.mult)
                    if debug:
                        dpr = stage.tile([128, 512], F32, tag="dbgpr",
                                         name=f"dpr_{r}")
                        nc.scalar.copy(dpr[:], pt[:])
                        nc.gpsimd.dma_start(dbg["pr"][r, :, :], dpr[:])
                        ssb = stage.tile([128, 512], F32, tag="ssb",
                                         name=f"ssb_{r}")
                        nc.scalar.copy(ssb[:], s_ps[:])
                        nc.gpsimd.dma_start(dbg["s"][r, :, :], ssb[:])
                    rowstate[rr] = pt

                def att_row_b(rr):
                    r = ti * rows_per_tile + rr
                    rp = rr % 2
                    pt = rowstate.pop(rr)

                    # v duplicated to both partition strips (SBUF->SBUF DMA)
                    vt = vts[rr // 2]
                    v2 = rowp.tile([128, DIM], BF16, tag="v2", name=f"v2_{r}")
                    nc.sync.dma_start(v2[0:64, :], vt[64 * rp:64 * rp + 64, :])
                    nc.sync.dma_start(v2[64:128, :], vt[64 * rp:64 * rp + 64, :])

                    # ctx: head pairs (even, odd) at partition strips (0, 64)
                    ctx_ps = att_ps.tile([128, 512], F32, tag="att",
                                         name=f"ctx_{r}")
                    for hp in range(8):
                        for par in range(2):
                            h = 2 * hp + par
                            st = 64 * par
                            nc.tensor.matmul(
                                ctx_ps[st:st + 64, 64 * hp:64 * hp + 64],
                                pt[st:st + 64, 64 * hp:64 * hp + 64],
                                v2[st:st + 64, 64 * h:64 * h + 64],
                                start=True, stop=True,
                                tile_position=(st, st),
                            )
                    osb = rowp.tile([128, 512], F32, tag="osb",
                                    name=f"osb_{r}", bufs=1)
                    nc.scalar.copy(osb[:], ctx_ps[:])
                    # out[64r + q, 64h + hd]; strip par holds heads 2hp+par
                    ov = out_d[M * r:M * r + M, :].rearrange(
                        "q (hp two hd) -> q hp two hd", two=2, hd=64)
                    for par in range(2):
                        nc.sync.dma_start(
                            ov[:, :, par, :],
                            osb[64 * par:64 * par + 64, :]
                            .rearrange("q (hp hd) -> q hp hd", hd=64),
                        )

                units = []
                for rr in range(rows_per_tile):
                    units.append(lambda rr=rr: att_row_a(rr))
                    units.append(lambda rr=rr: att_row_b(rr))
                return units

            prev_rows = []
            for ti in range(n_tiles):
                xt = xt0 if ti == 0 else emit_xt(ti)
                qt, kt, vts, groups = make_proj(ti, xt)
                ri = 0
                for gi, g in enumerate(groups):
                    g()
                    while (ri < len(prev_rows)
                           and (gi + 1) * len(prev_rows) // len(groups) > ri):
                        prev_rows[ri]()
                        ri += 1
                while ri < len(prev_rows):
                    prev_rows[ri]()
                    ri += 1
                prev_rows = make_att_rows(ti, qt, kt, vts)
            for row in prev_rows:
                row()

    return dict(out=out_d)


def _prepare_shards(hidden_states, attention_mask, sim_graph, Wq, bq, Wk, bk, Wv, bv,
                    n_cores=N_CORES):
    b, m, seq, dim = hidden_states.shape
    R = b * seq
    hs = np.transpose(np.asarray(hidden_states), (0, 2, 1, 3)).reshape(R, m, dim)
    am = np.ascontiguousarray(
        np.transpose(np.asarray(attention_mask), (0, 2, 1)).reshape(R, m),
        dtype=np.float32)
    sim = np.ascontiguousarray(
        np.transpose(np.asarray(sim_graph), (0, 2, 1)), dtype=np.float32)
    ident = np.eye(128, dtype=np.float32)
    selm = np.zeros((128, 2), np.float32)
    selm[0:64, 0] = 1.0
    selm[64:128, 1] = 1.0
    bselm = np.zeros((2, 128), np.float32)
    bselm[0, 0:64] = 1.0
    bselm[1, 64:128] = 1.0
    WqT = np.ascontiguousarray(np.asarray(Wq).T * 0.125, np.float32)
    WkT = np.ascontiguousarray(np.asarray(Wk).T, np.float32)
    WvT = np.ascontiguousarray(np.asarray(Wv).T, np.float32)
    rows_per_core = R // n_cores
    in_maps = []
    for c in range(n_cores):
        r0 = c * rows_per_core
        xT = np.ascontiguousarray(
            hs[r0:r0 + rows_per_core].reshape(rows_per_core * m, dim).T,
            np.float32)
        in_maps.append(dict(
            xT=xT,
            simg=sim[r0:r0 + rows_per_core],
            am=am[r0:r0 + rows_per_core],
            WqT=WqT, WkT=WkT, WvT=WvT,
            bq=np.ascontiguousarray(np.asarray(bq) * 0.125, np.float32),
            bk=np.ascontiguousarray(bk, np.float32),
            bv=np.ascontiguousarray(bv, np.float32),
            ident=ident, selm=selm, bselm=bselm,
        ))
    return in_maps


_CACHE = {}


def _get_compiled(use_bv=True):
    key = ("nc", use_bv)
    if key not in _CACHE:
        nc = bacc.Bacc("TRN2", target_bir_lowering=False, debug=False)
        build_core_kernel(nc, use_bv=use_bv)
        nc.compile()
        _CACHE[key] = nc
    return _CACHE[key]


LAST_EXEC_NS = [None]


def kernel(hidden_states, attention_mask, sim_graph, Wq, bq, Wk, bk, Wv, bv,
           b=4, m=64, seq=256, dim=1024, **_):
    import os
    from concourse.bass_utils import run_bass_kernel_spmd

    use_bv = bool(np.any(np.asarray(bv)))
    nc = _get_compiled(use_bv=use_bv)
    in_maps = _prepare_shards(hidden_states, attention_mask, sim_graph,
                              Wq, bq, Wk, bk, Wv, bv)
    trace = bool(int(os.environ.get("BERT_TRACE", "0")))
    if trace:
        try:  # register the NTFF hook if the middleware didn't
            from antenv.axon_hooks import (get_axon_ntff_profile_hook,
                                           set_axon_ntff_profile_hook)
            if get_axon_ntff_profile_hook() is None:
                from trn_agent_boot.trn_boot import _ntff_profile_via_ctypes
                set_axon_ntff_profile_hook(
                    _ntff_profile_via_ctypes("/opt/axon/libaxon_pjrt.so"))
        except Exception:
            trace = False
    res = run_bass_kernel_spmd(nc, in_maps, list(range(N_CORES)), trace=trace)
    LAST_EXEC_NS[0] = res.exec_time_ns
    R = int(b) * int(seq)
    out = np.concatenate([res.results[c]["out"] for c in range(N_CORES)], axis=0)
    return out.reshape(R, int(m), int(dim))


# revision 35
# speedup vs baseline: 1.5615x; 1.0318x over previous
"""Trainium2 Bass kernel for BertSimSelfAttention (sparse_attention).

Problem (full): B=4, M=64, SEQ=256, DIM=1024, H=16, HD=64.
Effective batch rows R = B*SEQ = 1024, each row: m=64 tokens of dim=1024.
  hs  = transpose(hidden_states,(0,2,1,3)).reshape(R, 64, 1024)
  q/k/v = hs @ W{q,k,v}.T + b   (per token)
  per (row, head): scores = (q @ k.T)/8 * sim[row] + (-1e4)*(1-am[row,j])
  probs = softmax_j(scores);  ctx = probs @ v  -> out [R, 64, 1024]

Sharding: data-parallel over rows, 128 rows/core x 8 cores. The host
pre-transposes x and W so the device consumes contraction-major layouts
directly (layout prep is part of the shard step).

Per-core design:
  - xT [d, t] and WT [d, o] loaded d-major, rounded to fp32r on DVE.
  - Projections in fp32r (1 cyc/row on PE): qT/kT [o, t] bf16
    (heads on partition strips by parity), v natural [t, o] bf16,
    masked by am and biased at evacuation.
  - scores per (row, head): bf16 paired matmuls via tile_position
    (head-even strips 0, head-odd 64) into one PSUM bank
    [128 = 2x64 q, 512 = 8 head-pairs x 64 j] (fp32).
  - softmax: t = S*sim (DVE, sim repeated via stride-0 AP), += mask
    (DVE; mask bcast built by identity-column matmul), exp (ACT),
    per-block reduce + reciprocal + normalize (DVE) -> probs bf16.
    No max-subtraction needed (|scores| <= ~8); masked lanes hit
    exp(-1e4) == 0 exactly.
  - probs transposed per head-pair ([128, 64] PE transposes) into two
    shared PSUM banks, row-parity selects the partition strip via
    tile_position; two [128, 512] evacuations per row-pair.
  - ctx: bf16 matmuls pairing (row_even, row_odd) per head so the
    natural v layout needs no duplication; PSUM -> SBUF (ACT) -> DRAM.
"""
import sys

sys.path.insert(0, "/opt/trn_rl_repo")

import numpy as np
import concourse.bass as bass
import concourse.bacc as bacc
import concourse.mybir as mybir
import concourse.tile as tile

F32 = mybir.dt.float32
F32R = mybir.dt.float32r
BF16 = mybir.dt.bfloat16
AF = mybir.ActivationFunctionType
ALU = mybir.AluOpType

N_CORES = 8
M = 64                    # tokens per row
DIM = 1024
H = 16
HD = 64
NEG = -10000.0


def build_core_kernel(nc, n_tiles=16, rows_per_tile=8, debug=False, use_bv=True):
    """Emit the per-core program. tile = rows_per_tile rows (must be even)."""
    T_TILE = rows_per_tile * M        # tokens per tile (512 default)
    n_rows = n_tiles * rows_per_tile
    n_tok = n_rows * M
    SUB = T_TILE // 128               # 128-token subtiles per tile

    xt_d = nc.dram_tensor("xT", (DIM, n_tok), F32, kind="ExternalInput")
    sim_d = nc.dram_tensor("simg", (n_rows, M, M), F32, kind="ExternalInput")
    am_d = nc.dram_tensor("am", (n_rows, M), F32, kind="ExternalInput")
    wq_d = nc.dram_tensor("WqT", (DIM, DIM), F32, kind="ExternalInput")
    wk_d = nc.dram_tensor("WkT", (DIM, DIM), F32, kind="ExternalInput")
    wv_d = nc.dram_tensor("WvT", (DIM, DIM), F32, kind="ExternalInput")
    bq_d = nc.dram_tensor("bq", (DIM,), F32, kind="ExternalInput")
    bk_d = nc.dram_tensor("bk", (DIM,), F32, kind="ExternalInput")
    bv_d = nc.dram_tensor("bv", (DIM,), F32, kind="ExternalInput")
    id_d = nc.dram_tensor("ident", (128, 128), F32, kind="ExternalInput")
    sel_d = nc.dram_tensor("selm", (128, 2), F32, kind="ExternalInput")
    bsel_d = nc.dram_tensor("bselm", (2, 128), F32, kind="ExternalInput")
    out_d = nc.dram_tensor("out", (n_tok, DIM), F32, kind="ExternalOutput")

    dbg = {}
    if debug:
        dbg["qt"] = nc.dram_tensor("dbg_qt", (DIM, n_tok), F32, kind="ExternalOutput")
        dbg["kt"] = nc.dram_tensor("dbg_kt", (DIM, n_tok), F32, kind="ExternalOutput")
        dbg["v"] = nc.dram_tensor("dbg_v", (n_tok, DIM), F32, kind="ExternalOutput")
        dbg["pr"] = nc.dram_tensor("dbg_pr", (n_rows, 128, 512), F32,
                                   kind="ExternalOutput")
        dbg["s"] = nc.dram_tensor("dbg_s", (n_rows, 128, 512), F32,
                                  kind="ExternalOutput")

    with tile.TileContext(nc) as tc:
        with (
            tc.tile_pool(name="consts", bufs=1) as consts,
            tc.tile_pool(name="stage", bufs=3) as stage,
            tc.tile_pool(name="xtp", bufs=2) as xtp,
            tc.tile_pool(name="qkp", bufs=2) as qkp,
            tc.tile_pool(name="vp", bufs=2) as vp,
            tc.tile_pool(name="rowp", bufs=2) as rowp,
            tc.tile_pool(name="small_ps", bufs=2, space="PSUM") as small_ps,
            tc.tile_pool(name="proj_ps", bufs=2, space="PSUM") as proj_ps,
            tc.tile_pool(name="att_ps", bufs=4, space="PSUM") as att_ps,
        ):
            # ---------------- tiny consts first ----------------
            ident = consts.tile([128, 128], F32)
            nc.sync.dma_start(ident[:], id_d[:])

            am_all = consts.tile([128, M], F32)
            if n_rows < 128:
                nc.gpsimd.memset(am_all[:], 1.0)
            nc.sync.dma_start(am_all[0:n_rows, :], am_d[:])

            # block-ones selector: halfones[p, m] = 1 iff same 64-half;
            # halfones.T @ e sums each half and broadcasts to its partitions
            halfones = consts.tile([128, 128], BF16)
            nc.gpsimd.memset(halfones[:], 0.0)
            nc.gpsimd.memset(halfones[0:64, 0:64], 1.0)
            nc.gpsimd.memset(halfones[64:128, 64:128], 1.0)

            bq_sb = consts.tile([128, 8], F32)
            bk_sb = consts.tile([128, 8], F32)
            nc.sync.dma_start(bq_sb[:], bq_d[:].rearrange("(o p) -> p o", p=128))
            nc.sync.dma_start(bk_sb[:], bk_d[:].rearrange("(o p) -> p o", p=128))

            if use_bv:
                # bv as a K=1 fp32r pair for psum-accumulate
                ones_f = consts.tile([1, 128], F32)
                nc.gpsimd.memset(ones_f[:], 1.0)
                ones_r = consts.tile([1, 128], F32R)
                nc.vector.tensor_copy(ones_r[:], ones_f[:])
                bv_row = consts.tile([1, DIM], F32)
                nc.sync.dma_start(bv_row[:],
                                  bv_d[:].rearrange("(a o) -> a o", a=1))
                bv_r = consts.tile([1, DIM], F32R)
                nc.vector.tensor_copy(bv_r[:], bv_row[:])

            # mask bias columns: mcolT2[:, r] = -1e4*(1 - am[r, j]) on both
            # partition halves (exp-bias per key token j)
            mcolT2 = consts.tile([128, 128], F32)
            amt_ps = small_ps.tile([128, 128], F32, tag="srb")
            nc.tensor.transpose(amt_ps[0:M, 0:128], am_all[:], ident[:])
            nc.vector.tensor_scalar(
                mcolT2[0:64, :], amt_ps[0:M, 0:128], -NEG, NEG,
                op0=ALU.mult, op1=ALU.add)
            nc.vector.tensor_scalar(
                mcolT2[64:128, :], amt_ps[0:M, 0:128], -NEG, NEG,
                op0=ALU.mult, op1=ALU.add)

            # ---------------- weights (+ tile-0 x interleaved) ----------
            def emit_xt(ti):
                t0 = ti * T_TILE
                xt = [xtp.tile([128, T_TILE], F32R, tag=f"xt{d}",
                               name=f"xt{d}_{ti}") for d in range(8)]
                for dch in range(8):
                    xst = stage.tile([128, T_TILE], F32, tag="xstage",
                                     name=f"xst{dch}_{ti}")
                    nc.sync.dma_start(
                        xst[:], xt_d[128 * dch:128 * dch + 128, t0:t0 + T_TILE]
                    )
                    nc.vector.tensor_copy(xt[dch][:], xst[:])
                return xt

            def emit_w(name, w_d, dchs):
                wt = wts[name]
                for dch in dchs:
                    for hh in range(DIM // 512):
                        wnat = stage.tile([128, 512], F32, tag="xstage",
                                          name=f"wn{name}{dch}{hh}")
                        nc.gpsimd.dma_start(
                            wnat[:],
                            w_d[128 * dch:128 * dch + 128,
                                512 * hh:512 * hh + 512])
                        nc.vector.tensor_copy(
                            wt[dch][:, 512 * hh:512 * hh + 512], wnat[:])

            wts = {name: [consts.tile([128, DIM], F32R, tag=f"w{name}{d}",
                                      name=f"w{name}{d}") for d in range(8)]
                   for name in ("q", "k", "v")}
            xt0 = [xtp.tile([128, T_TILE], F32R, tag=f"xt{d}",
                             name=f"xt{d}_0") for d in range(8)]
            for dch in range(8):
                emit_w("q", wq_d, [dch])
                xst = stage.tile([128, T_TILE], F32, tag="xstage",
                                 name=f"xst{dch}_0")
                nc.sync.dma_start(xst[:], xt_d[128 * dch:128 * dch + 128,
                                               0:T_TILE])
                nc.vector.tensor_copy(xt0[dch][:], xst[:])
            emit_w("k", wk_d, range(8))
            emit_w("v", wv_d, range(8))
            wqt, wkt, wvt = wts["q"], wts["k"], wts["v"]

            # ---------------- main loop over token tiles ----------------
            # Emission interleaves tile ti's projection groups with tile
            # (ti-1)'s attention rows so the PE program order has dense
            # matmul work to fill softmax dependency stalls (keeps HAM warm).

            def make_proj(ti, xt):
                qt = [qkp.tile([128, T_TILE], BF16, tag=f"qt{o}",
                               name=f"qt{o}_{ti}") for o in range(8)]
                kt = [qkp.tile([128, T_TILE], BF16, tag=f"kt{o}",
                               name=f"kt{o}_{ti}") for o in range(8)]
                vts = [vp.tile([128, DIM], BF16, tag=f"v{s}",
                               name=f"v{s}_{ti}") for s in range(SUB)]
                groups = []

                def qk_group(wt, dst, b_sb, och):
                    ps = proj_ps.tile([128, T_TILE], F32, tag="proj",
                                      name=f"qkps{och}_{ti}")
                    for dch in range(8):
                        nc.tensor.matmul(
                            ps[:],
                            wt[dch][:, 128 * och:128 * och + 128],
                            xt[dch][:],
                            start=(dch == 0), stop=(dch == 7),
                        )
                    nc.scalar.activation(
                        dst[och][:], ps[:], AF.Identity,
                        bias=b_sb[:, och:och + 1], scale=1.0,
                    )

                def v_group(sub, oh):
                    vt = vts[sub]
                    ps = proj_ps.tile([128, 512], F32, tag="proj",
                                      name=f"vps{sub}{oh}_{ti}")
                    sl = slice(512 * oh, 512 * oh + 512)
                    for dch in range(8):
                        nc.tensor.matmul(
                            ps[:],
                            xt[dch][:, 128 * sub:128 * sub + 128],
                            wvt[dch][:, 512 * oh:512 * oh + 512],
                            start=(dch == 0), stop=(dch == 7) and not use_bv,
                        )
                    if use_bv:
                        nc.tensor.matmul(
                            ps[:], ones_r[:], bv_r[:, sl],
                            start=False, stop=True,
                        )
                    nc.scalar.copy(vt[:, sl], ps[:])

                for wt, dst, b_sb in ((wqt, qt, bq_sb), (wkt, kt, bk_sb)):
                    for och in range(8):
                        groups.append(
                            lambda wt=wt, dst=dst, b_sb=b_sb, och=och:
                            qk_group(wt, dst, b_sb, och))
                for sub in range(SUB):
                    for oh in range(2):
                        groups.append(lambda sub=sub, oh=oh: v_group(sub, oh))
                return qt, kt, vts, groups

            def make_att_rows(ti, qt, kt, vts):
                rowstate = {}

                def att_row_a(rr):
                    r = ti * rows_per_tile + rr

                    simt2 = rowp.tile([128, M], F32, tag="sim2",
                                      name=f"sim2_{r}")
                    nc.sync.dma_start(simt2[0:64, :], sim_d[r, :, :])
                    nc.sync.dma_start(simt2[64:128, :], sim_d[r, :, :])

                    # scores transposed: S'[j, q] (bf16 in, fp32 psum)
                    s_ps = att_ps.tile([128, 512], F32, tag="att",
                                       name=f"s_{r}")
                    tsl = slice(M * rr, M * rr + M)
                    for h in range(H):
                        hp, half = h // 2, h % 2
                        st = 64 * half
                        nc.tensor.matmul(
                            s_ps[st:st + 64, 64 * hp:64 * hp + 64],
                            kt[h // 2][st:st + 64, tsl],
                            qt[h // 2][st:st + 64, tsl],
                            start=True, stop=True,
                            tile_position=(st, st),
                        )

                    # t = S' * simT;  e = exp(t + maskcol)  (bf16 out)
                    tt = rowp.tile([128, 512], F32, tag="tt", name=f"tt_{r}")
                    nc.vector.tensor_tensor(
                        tt[:].rearrange("p (a j) -> p a j", j=M),
                        s_ps[:].rearrange("p (a j) -> p a j", j=M),
                        simt2[:].rearrange("p (a j) -> p a j", a=1)
                        .broadcast_to([128, 8, M]),
                        op=ALU.mult,
                    )
                    et = rowp.tile([128, 512], BF16, tag="et", name=f"et_{r}")
                    nc.scalar.activation(et[:], tt[:], AF.Exp,
                                         bias=mcolT2[:, r:r + 1])

                    # denominators summed + broadcast in one PE matmul
                    dn_ps = small_ps.tile([128, 512], F32, tag="srb",
                                          name=f"dn_{r}")
                    nc.tensor.matmul(dn_ps[:], halfones[:], et[:],
                                     start=True, stop=True)

                    # in-place approx reciprocal on PSUM (~18 bits), then
                    # probs.T = e * (1/denom) (bf16) -- direct ctx stationary
                    nc.vector.reciprocal_approx_fast(out=dn_ps[:], in_=dn_ps[:])
                    pt = rowp.tile([128, 512], BF16, tag="pt", name=f"pt_{r}")
                    nc.vector.tensor_tensor(pt[:], et[:], dn_ps[:],
                                            op=ALU.mult)
                    if debug:
                        dpr = stage.tile([128, 512], F32, tag="dbgpr",
                                         name=f"dpr_{r}")
                        nc.scalar.copy(dpr[:], pt[:])
                        nc.gpsimd.dma_start(dbg["pr"][r, :, :], dpr[:])
                        ssb = stage.tile([128, 512], F32, tag="ssb",
                                         name=f"ssb_{r}")
                        nc.scalar.copy(ssb[:], s_ps[:])
                        nc.gpsimd.dma_start(dbg["s"][r, :, :], ssb[:])
                    rowstate[rr] = pt

                def att_row_b(rr):
                    r = ti * rows_per_tile + rr
                    rp = rr % 2
                    pt = rowstate.pop(rr)

                    # v duplicated to both partition strips (SBUF->SBUF DMA)
                    vt = vts[rr // 2]
                    v2 = rowp.tile([128, DIM], BF16, tag="v2", name=f"v2_{r}")
                    nc.sync.dma_start(v2[0:64, :], vt[64 * rp:64 * rp + 64, :])
                    nc.sync.dma_start(v2[64:128, :], vt[64 * rp:64 * rp + 64, :])

                    # ctx: head pairs (even, odd) at partition strips (0, 64)
                    ctx_ps = att_ps.tile([128, 512], F32, tag="att",
                                         name=f"ctx_{r}")
                    for hp in range(8):
                        for par in range(2):
                            h = 2 * hp + par
                            st = 64 * par
                            nc.tensor.matmul(
                                ctx_ps[st:st + 64, 64 * hp:64 * hp + 64],
                                pt[st:st + 64, 64 * hp:64 * hp + 64],
                                v2[st:st + 64, 64 * h:64 * h + 64],
                                start=True, stop=True,
                                tile_position=(st, st),
                            )
                    osb = rowp.tile([128, 512], F32, tag="osb",
                                    name=f"osb_{r}", bufs=1)
                    nc.vector.tensor_copy(osb[:], ctx_ps[:])
                    # out[64r + q, 64h + hd]; strip par holds heads 2hp+par
                    ov = out_d[M * r:M * r + M, :].rearrange(
                        "q (hp two hd) -> q hp two hd", two=2, hd=64)
                    for par in range(2):
                        nc.sync.dma_start(
                            ov[:, :, par, :],
                            osb[64 * par:64 * par + 64, :]
                            .rearrange("q (hp hd) -> q hp hd", hd=64),
                        )

                units = []
                for rr in range(rows_per_tile):
                    units.append(lambda rr=rr: att_row_a(rr))
                    units.append(lambda rr=rr: att_row_b(rr))
                return units

            prev_rows = []
            for ti in range(n_tiles):
                xt = xt0 if ti == 0 else emit_xt(ti)
                qt, kt, vts, groups = make_proj(ti, xt)
                ri = 0
                for gi, g in enumerate(groups):
                    g()
                    while (ri < len(prev_rows)
                           and (gi + 1) * len(prev_rows) // len(groups) > ri):
                        prev_rows[ri]()
                        ri += 1
                while ri < len(prev_rows):
                    prev_rows[ri]()
                    ri += 1
                prev_rows = make_att_rows(ti, qt, kt, vts)
            for row in prev_rows:
                row()

    return dict(out=out_d)


def _prepare_shards(hidden_states, attention_mask, sim_graph, Wq, bq, Wk, bk, Wv, bv,
                    n_cores=N_CORES):
    b, m, seq, dim = hidden_states.shape
    R = b * seq
    hs = np.transpose(np.asarray(hidden_states), (0, 2, 1, 3)).reshape(R, m, dim)
    am = np.ascontiguousarray(
        np.transpose(np.asarray(attention_mask), (0, 2, 1)).reshape(R, m),
        dtype=np.float32)
    sim = np.ascontiguousarray(
        np.transpose(np.asarray(sim_graph), (0, 2, 1)), dtype=np.float32)
    ident = np.eye(128, dtype=np.float32)
    selm = np.zeros((128, 2), np.float32)
    selm[0:64, 0] = 1.0
    selm[64:128, 1] = 1.0
    bselm = np.zeros((2, 128), np.float32)
    bselm[0, 0:64] = 1.0
    bselm[1, 64:128] = 1.0
    WqT = np.ascontiguousarray(np.asarray(Wq).T * 0.125, np.float32)
    WkT = np.ascontiguousarray(np.asarray(Wk).T, np.float32)
    WvT = np.ascontiguousarray(np.asarray(Wv).T, np.float32)
    rows_per_core = R // n_cores
    in_maps = []
    for c in range(n_cores):
        r0 = c * rows_per_core
        xT = np.ascontiguousarray(
            hs[r0:r0 + rows_per_core].reshape(rows_per_core * m, dim).T,
            np.float32)
        in_maps.append(dict(
            xT=xT,
            simg=sim[r0:r0 + rows_per_core],
            am=am[r0:r0 + rows_per_core],
            WqT=WqT, WkT=WkT, WvT=WvT,
            bq=np.ascontiguousarray(np.asarray(bq) * 0.125, np.float32),
            bk=np.ascontiguousarray(bk, np.float32),
            bv=np.ascontiguousarray(bv, np.float32),
            ident=ident, selm=selm, bselm=bselm,
        ))
    return in_maps


_CACHE = {}


def _get_compiled(use_bv=True):
    key = ("nc", use_bv)
    if key not in _CACHE:
        nc = bacc.Bacc("TRN2", target_bir_lowering=False, debug=False)
        build_core_kernel(nc, use_bv=use_bv)
        nc.compile()
        _CACHE[key] = nc
    return _CACHE[key]


LAST_EXEC_NS = [None]


def kernel(hidden_states, attention_mask, sim_graph, Wq, bq, Wk, bk, Wv, bv,
           b=4, m=64, seq=256, dim=1024, **_):
    import os
    from concourse.bass_utils import run_bass_kernel_spmd

    use_bv = bool(np.any(np.asarray(bv)))
    nc = _get_compiled(use_bv=use_bv)
    in_maps = _prepare_shards(hidden_states, attention_mask, sim_graph,
                              Wq, bq, Wk, bk, Wv, bv)
    trace = bool(int(os.environ.get("BERT_TRACE", "0")))
    if trace:
        try:  # register the NTFF hook if the middleware didn't
            from antenv.axon_hooks import (get_axon_ntff_profile_hook,
                                           set_axon_ntff_profile_hook)
            if get_axon_ntff_profile_hook() is None:
                from trn_agent_boot.trn_boot import _ntff_profile_via_ctypes
                set_axon_ntff_profile_hook(
                    _ntff_profile_via_ctypes("/opt/axon/libaxon_pjrt.so"))
        except Exception:
            trace = False
    res = run_bass_kernel_spmd(nc, in_maps, list(range(N_CORES)), trace=trace)
    LAST_EXEC_NS[0] = res.exec_time_ns
    R = int(b) * int(seq)
    out = np.concatenate([res.results[c]["out"] for c in range(N_CORES)], axis=0)
    return out.reshape(R, int(m), int(dim))


# revision 36
# speedup vs baseline: 1.5955x; 1.0218x over previous
"""Trainium2 Bass kernel for BertSimSelfAttention (sparse_attention).

Problem (full): B=4, M=64, SEQ=256, DIM=1024, H=16, HD=64.
Effective batch rows R = B*SEQ = 1024, each row: m=64 tokens of dim=1024.
  hs  = transpose(hidden_states,(0,2,1,3)).reshape(R, 64, 1024)
  q/k/v = hs @ W{q,k,v}.T + b   (per token)
  per (row, head): scores = (q @ k.T)/8 * sim[row] + (-1e4)*(1-am[row,j])
  probs = softmax_j(scores);  ctx = probs @ v  -> out [R, 64, 1024]

Sharding: data-parallel over rows, 128 rows/core x 8 cores. The host
pre-transposes x and W so the device consumes contraction-major layouts
directly (layout prep is part of the shard step).

Per-core design:
  - xT [d, t] and WT [d, o] loaded d-major, rounded to fp32r on DVE.
  - Projections in fp32r (1 cyc/row on PE): qT/kT [o, t] bf16
    (heads on partition strips by parity), v natural [t, o] bf16,
    masked by am and biased at evacuation.
  - scores per (row, head): bf16 paired matmuls via tile_position
    (head-even strips 0, head-odd 64) into one PSUM bank
    [128 = 2x64 q, 512 = 8 head-pairs x 64 j] (fp32).
  - softmax: t = S*sim (DVE, sim repeated via stride-0 AP), += mask
    (DVE; mask bcast built by identity-column matmul), exp (ACT),
    per-block reduce + reciprocal + normalize (DVE) -> probs bf16.
    No max-subtraction needed (|scores| <= ~8); masked lanes hit
    exp(-1e4) == 0 exactly.
  - probs transposed per head-pair ([128, 64] PE transposes) into two
    shared PSUM banks, row-parity selects the partition strip via
    tile_position; two [128, 512] evacuations per row-pair.
  - ctx: bf16 matmuls pairing (row_even, row_odd) per head so the
    natural v layout needs no duplication; PSUM -> SBUF (ACT) -> DRAM.
"""
import sys

sys.path.insert(0, "/opt/trn_rl_repo")

import numpy as np
import concourse.bass as bass
import concourse.bacc as bacc
import concourse.mybir as mybir
import concourse.tile as tile

F32 = mybir.dt.float32
F32R = mybir.dt.float32r
BF16 = mybir.dt.bfloat16
AF = mybir.ActivationFunctionType
ALU = mybir.AluOpType

N_CORES = 8
M = 64                    # tokens per row
DIM = 1024
H = 16
HD = 64
NEG = -10000.0


def build_core_kernel(nc, n_tiles=16, rows_per_tile=8, debug=False, use_bv=True):
    """Emit the per-core program. tile = rows_per_tile rows (must be even)."""
    T_TILE = rows_per_tile * M        # tokens per tile (512 default)
    n_rows = n_tiles * rows_per_tile
    n_tok = n_rows * M
    SUB = T_TILE // 128               # 128-token subtiles per tile

    xt_d = nc.dram_tensor("xT", (DIM, n_tok), F32, kind="ExternalInput")
    sim_d = nc.dram_tensor("simg", (n_rows, M, M), F32, kind="ExternalInput")
    am_d = nc.dram_tensor("am", (n_rows, M), F32, kind="ExternalInput")
    wq_d = nc.dram_tensor("WqT", (DIM, DIM), F32, kind="ExternalInput")
    wk_d = nc.dram_tensor("WkT", (DIM, DIM), F32, kind="ExternalInput")
    wv_d = nc.dram_tensor("WvT", (DIM, DIM), F32, kind="ExternalInput")
    bq_d = nc.dram_tensor("bq", (DIM,), F32, kind="ExternalInput")
    bk_d = nc.dram_tensor("bk", (DIM,), F32, kind="ExternalInput")
    bv_d = nc.dram_tensor("bv", (DIM,), F32, kind="ExternalInput")
    id_d = nc.dram_tensor("ident", (128, 128), F32, kind="ExternalInput")
    sel_d = nc.dram_tensor("selm", (128, 2), F32, kind="ExternalInput")
    bsel_d = nc.dram_tensor("bselm", (2, 128), F32, kind="ExternalInput")
    out_d = nc.dram_tensor("out", (n_tok, DIM), F32, kind="ExternalOutput")

    dbg = {}
    if debug:
        dbg["qt"] = nc.dram_tensor("dbg_qt", (DIM, n_tok), F32, kind="ExternalOutput")
        dbg["kt"] = nc.dram_tensor("dbg_kt", (DIM, n_tok), F32, kind="ExternalOutput")
        dbg["v"] = nc.dram_tensor("dbg_v", (n_tok, DIM), F32, kind="ExternalOutput")
        dbg["pr"] = nc.dram_tensor("dbg_pr", (n_rows, 128, 512), F32,
                                   kind="ExternalOutput")
        dbg["s"] = nc.dram_tensor("dbg_s", (n_rows, 128, 512), F32,
                                  kind="ExternalOutput")

    with tile.TileContext(nc) as tc:
        with (
            tc.tile_pool(name="consts", bufs=1) as consts,
            tc.tile_pool(name="stage", bufs=3) as stage,
            tc.tile_pool(name="xtp", bufs=2) as xtp,
            tc.tile_pool(name="qkp", bufs=2) as qkp,
            tc.tile_pool(name="vp", bufs=2) as vp,
            tc.tile_pool(name="rowp", bufs=2) as rowp,
            tc.tile_pool(name="small_ps", bufs=2, space="PSUM") as small_ps,
            tc.tile_pool(name="proj_ps", bufs=2, space="PSUM") as proj_ps,
            tc.tile_pool(name="att_ps", bufs=4, space="PSUM") as att_ps,
        ):
            # ---------------- tiny consts first ----------------
            ident = consts.tile([128, 128], F32)
            nc.sync.dma_start(ident[:], id_d[:])

            am_all = consts.tile([128, M], F32)
            if n_rows < 128:
                nc.gpsimd.memset(am_all[:], 1.0)
            nc.sync.dma_start(am_all[0:n_rows, :], am_d[:])

            # block-ones selector: halfones[p, m] = 1 iff same 64-half;
            # halfones.T @ e sums each half and broadcasts to its partitions
            halfones = consts.tile([128, 128], BF16)
            nc.gpsimd.memset(halfones[:], 0.0)
            nc.gpsimd.memset(halfones[0:64, 0:64], 1.0)
            nc.gpsimd.memset(halfones[64:128, 64:128], 1.0)

            bq_sb = consts.tile([128, 8], F32)
            bk_sb = consts.tile([128, 8], F32)
            nc.sync.dma_start(bq_sb[:], bq_d[:].rearrange("(o p) -> p o", p=128))
            nc.sync.dma_start(bk_sb[:], bk_d[:].rearrange("(o p) -> p o", p=128))

            if use_bv:
                # bv as a K=1 fp32r pair for psum-accumulate
                ones_f = consts.tile([1, 128], F32)
                nc.gpsimd.memset(ones_f[:], 1.0)
                ones_r = consts.tile([1, 128], F32R)
                nc.vector.tensor_copy(ones_r[:], ones_f[:])
                bv_row = consts.tile([1, DIM], F32)
                nc.sync.dma_start(bv_row[:],
                                  bv_d[:].rearrange("(a o) -> a o", a=1))
                bv_r = consts.tile([1, DIM], F32R)
                nc.vector.tensor_copy(bv_r[:], bv_row[:])

            # mask bias columns: mcolT2[:, r] = -1e4*(1 - am[r, j]) on both
            # partition halves (exp-bias per key token j)
            mcolT2 = consts.tile([128, 128], F32)
            amt_ps = small_ps.tile([128, 128], F32, tag="srb")
            nc.tensor.transpose(amt_ps[0:M, 0:128], am_all[:], ident[:])
            nc.vector.tensor_scalar(
                mcolT2[0:64, :], amt_ps[0:M, 0:128], -NEG, NEG,
                op0=ALU.mult, op1=ALU.add)
            nc.vector.tensor_scalar(
                mcolT2[64:128, :], amt_ps[0:M, 0:128], -NEG, NEG,
                op0=ALU.mult, op1=ALU.add)

            # ---------------- weights (+ tile-0 x interleaved) ----------
            def emit_xt(ti):
                t0 = ti * T_TILE
                xt = [xtp.tile([128, T_TILE], F32R, tag=f"xt{d}",
                               name=f"xt{d}_{ti}") for d in range(8)]
                for dch in range(8):
                    xst = stage.tile([128, T_TILE], F32, tag="xstage",
                                     name=f"xst{dch}_{ti}")
                    nc.sync.dma_start(
                        xst[:], xt_d[128 * dch:128 * dch + 128, t0:t0 + T_TILE]
                    )
                    nc.vector.tensor_copy(xt[dch][:], xst[:])
                return xt

            def emit_w(name, w_d, dchs):
                wt = wts[name]
                for dch in dchs:
                    for hh in range(DIM // 512):
                        wnat = stage.tile([128, 512], F32, tag="xstage",
                                          name=f"wn{name}{dch}{hh}")
                        nc.gpsimd.dma_start(
                            wnat[:],
                            w_d[128 * dch:128 * dch + 128,
                                512 * hh:512 * hh + 512])
                        nc.vector.tensor_copy(
                            wt[dch][:, 512 * hh:512 * hh + 512], wnat[:])

            wts = {name: [consts.tile([128, DIM], F32R, tag=f"w{name}{d}",
                                      name=f"w{name}{d}") for d in range(8)]
                   for name in ("q", "k", "v")}
            xt0 = [xtp.tile([128, T_TILE], F32R, tag=f"xt{d}",
                             name=f"xt{d}_0") for d in range(8)]
            for dch in range(8):
                emit_w("q", wq_d, [dch])
                xst = stage.tile([128, T_TILE], F32, tag="xstage",
                                 name=f"xst{dch}_0")
                nc.sync.dma_start(xst[:], xt_d[128 * dch:128 * dch + 128,
                                               0:T_TILE])
                nc.vector.tensor_copy(xt0[dch][:], xst[:])
            emit_w("k", wk_d, range(8))
            emit_w("v", wv_d, range(8))
            wqt, wkt, wvt = wts["q"], wts["k"], wts["v"]

            # ---------------- main loop over token tiles ----------------
            # Emission interleaves tile ti's projection groups with tile
            # (ti-1)'s attention rows so the PE program order has dense
            # matmul work to fill softmax dependency stalls (keeps HAM warm).

            def make_proj(ti, xt):
                qt = [qkp.tile([128, T_TILE], BF16, tag=f"qt{o}",
                               name=f"qt{o}_{ti}") for o in range(8)]
                kt = [qkp.tile([128, T_TILE], BF16, tag=f"kt{o}",
                               name=f"kt{o}_{ti}") for o in range(8)]
                vts = [vp.tile([128, DIM], BF16, tag=f"v{s}",
                               name=f"v{s}_{ti}") for s in range(SUB)]
                groups = []

                def qk_group(wt, dst, b_sb, och):
                    ps = proj_ps.tile([128, T_TILE], F32, tag="proj",
                                      name=f"qkps{och}_{ti}")
                    for dch in range(8):
                        nc.tensor.matmul(
                            ps[:],
                            wt[dch][:, 128 * och:128 * och + 128],
                            xt[dch][:],
                            start=(dch == 0), stop=(dch == 7),
                        )
                    nc.scalar.activation(
                        dst[och][:], ps[:], AF.Identity,
                        bias=b_sb[:, och:och + 1], scale=1.0,
                    )

                def v_group(sub, oh):
                    vt = vts[sub]
                    ps = proj_ps.tile([128, 512], F32, tag="proj",
                                      name=f"vps{sub}{oh}_{ti}")
                    sl = slice(512 * oh, 512 * oh + 512)
                    for dch in range(8):
                        nc.tensor.matmul(
                            ps[:],
                            xt[dch][:, 128 * sub:128 * sub + 128],
                            wvt[dch][:, 512 * oh:512 * oh + 512],
                            start=(dch == 0), stop=(dch == 7) and not use_bv,
                        )
                    if use_bv:
                        nc.tensor.matmul(
                            ps[:], ones_r[:], bv_r[:, sl],
                            start=False, stop=True,
                        )
                    nc.scalar.copy(vt[:, sl], ps[:])

                for wt, dst, b_sb in ((wqt, qt, bq_sb), (wkt, kt, bk_sb)):
                    for och in range(8):
                        groups.append(
                            lambda wt=wt, dst=dst, b_sb=b_sb, och=och:
                            qk_group(wt, dst, b_sb, och))
                for sub in range(SUB):
                    for oh in range(2):
                        groups.append(lambda sub=sub, oh=oh: v_group(sub, oh))
                return qt, kt, vts, groups

            def make_att_rows(ti, qt, kt, vts):
                rowstate = {}

                def att_row_a(rr):
                    r = ti * rows_per_tile + rr

                    simt2 = rowp.tile([128, M], F32, tag="sim2",
                                      name=f"sim2_{r}")
                    nc.sync.dma_start(simt2[0:64, :], sim_d[r, :, :])
                    nc.sync.dma_start(simt2[64:128, :], sim_d[r, :, :])

                    # scores transposed: S'[j, q] (bf16 in, fp32 psum)
                    s_ps = att_ps.tile([128, 512], F32, tag="att",
                                       name=f"s_{r}")
                    tsl = slice(M * rr, M * rr + M)
                    for h in range(H):
                        hp, half = h // 2, h % 2
                        st = 64 * half
                        nc.tensor.matmul(
                            s_ps[st:st + 64, 64 * hp:64 * hp + 64],
                            kt[h // 2][st:st + 64, tsl],
                            qt[h // 2][st:st + 64, tsl],
                            start=True, stop=True,
                            tile_position=(st, st),
                        )

                    # t = S' * simT;  e = exp(t + maskcol)  (bf16 out)
                    tt = rowp.tile([128, 512], F32, tag="tt", name=f"tt_{r}")
                    nc.vector.tensor_tensor(
                        tt[:].rearrange("p (a j) -> p a j", j=M),
                        s_ps[:].rearrange("p (a j) -> p a j", j=M),
                        simt2[:].rearrange("p (a j) -> p a j", a=1)
                        .broadcast_to([128, 8, M]),
                        op=ALU.mult,
                    )
                    et = rowp.tile([128, 512], BF16, tag="et", name=f"et_{r}")
                    nc.scalar.activation(et[:], tt[:], AF.Exp,
                                         bias=mcolT2[:, r:r + 1])

                    # denominators summed + broadcast in one PE matmul
                    dn_ps = small_ps.tile([128, 512], F32, tag="srb",
                                          name=f"dn_{r}")
                    nc.tensor.matmul(dn_ps[:], halfones[:], et[:],
                                     start=True, stop=True)

                    # in-place approx reciprocal on PSUM (~18 bits), then
                    # probs.T = e * (1/denom) (bf16) -- direct ctx stationary
                    nc.vector.reciprocal_approx_fast(out=dn_ps[:], in_=dn_ps[:])
                    pt = rowp.tile([128, 512], BF16, tag="pt", name=f"pt_{r}")
                    nc.vector.tensor_tensor(pt[:], et[:], dn_ps[:],
                                            op=ALU.mult)
                    if debug:
                        dpr = stage.tile([128, 512], F32, tag="dbgpr",
                                         name=f"dpr_{r}")
                        nc.scalar.copy(dpr[:], pt[:])
                        nc.gpsimd.dma_start(dbg["pr"][r, :, :], dpr[:])
                        ssb = stage.tile([128, 512], F32, tag="ssb",
                                         name=f"ssb_{r}")
                        nc.scalar.copy(ssb[:], s_ps[:])
                        nc.gpsimd.dma_start(dbg["s"][r, :, :], ssb[:])
                    rowstate[rr] = pt

                def att_row_b(rr):
                    r = ti * rows_per_tile + rr
                    rp = rr % 2
                    pt = rowstate.pop(rr)

                    # v duplicated to both partition strips (SBUF->SBUF DMA)
                    vt = vts[rr // 2]
                    v2 = rowp.tile([128, DIM], BF16, tag="v2", name=f"v2_{r}")
                    nc.sync.dma_start(v2[0:64, :], vt[64 * rp:64 * rp + 64, :])
                    nc.sync.dma_start(v2[64:128, :], vt[64 * rp:64 * rp + 64, :])

                    # ctx: head pairs (even, odd) at partition strips (0, 64)
                    ctx_ps = att_ps.tile([128, 512], F32, tag="att",
                                         name=f"ctx_{r}")
                    for hp in range(8):
                        for par in range(2):
                            h = 2 * hp + par
                            st = 64 * par
                            nc.tensor.matmul(
                                ctx_ps[st:st + 64, 64 * hp:64 * hp + 64],
                                pt[st:st + 64, 64 * hp:64 * hp + 64],
                                v2[st:st + 64, 64 * h:64 * h + 64],
                                start=True, stop=True,
                                tile_position=(st, st),
                            )
                    osb = rowp.tile([128, 512], F32, tag="osb",
                                    name=f"osb_{r}", bufs=1)
                    nc.vector.tensor_copy(osb[:], ctx_ps[:])
                    # out[64r + q, 64h + hd]; strip par holds heads 2hp+par
                    ov = out_d[M * r:M * r + M, :].rearrange(
                        "q (hp two hd) -> q hp two hd", two=2, hd=64)
                    for par in range(2):
                        nc.sync.dma_start(
                            ov[:, :, par, :],
                            osb[64 * par:64 * par + 64, :]
                            .rearrange("q (hp hd) -> q hp hd", hd=64),
                        )

                # lookahead-1 pipeline: A0 A1 B0 A2 B1 ... A7 B6 B7 so each
                # row's ctx matmuls can fill the next row's softmax stalls
                units = [lambda: att_row_a(0)]
                for rr in range(1, rows_per_tile):
                    units.append(lambda rr=rr: att_row_a(rr))
                    units.append(lambda rr=rr: att_row_b(rr - 1))
                units.append(lambda: att_row_b(rows_per_tile - 1))
                return units

            prev_rows = []
            for ti in range(n_tiles):
                xt = xt0 if ti == 0 else emit_xt(ti)
                qt, kt, vts, groups = make_proj(ti, xt)
                ri = 0
                for gi, g in enumerate(groups):
                    g()
                    while (ri < len(prev_rows)
                           and (gi + 1) * len(prev_rows) // len(groups) > ri):
                        prev_rows[ri]()
                        ri += 1
                while ri < len(prev_rows):
                    prev_rows[ri]()
                    ri += 1
                prev_rows = make_att_rows(ti, qt, kt, vts)
            for row in prev_rows:
                row()

    return dict(out=out_d)


def _prepare_shards(hidden_states, attention_mask, sim_graph, Wq, bq, Wk, bk, Wv, bv,
                    n_cores=N_CORES):
    b, m, seq, dim = hidden_states.shape
    R = b * seq
    hs = np.transpose(np.asarray(hidden_states), (0, 2, 1, 3)).reshape(R, m, dim)
    am = np.ascontiguousarray(
        np.transpose(np.asarray(attention_mask), (0, 2, 1)).reshape(R, m),
        dtype=np.float32)
    sim = np.ascontiguousarray(
        np.transpose(np.asarray(sim_graph), (0, 2, 1)), dtype=np.float32)
    ident = np.eye(128, dtype=np.float32)
    selm = np.zeros((128, 2), np.float32)
    selm[0:64, 0] = 1.0
    selm[64:128, 1] = 1.0
    bselm = np.zeros((2, 128), np.float32)
    bselm[0, 0:64] = 1.0
    bselm[1, 64:128] = 1.0
    WqT = np.ascontiguousarray(np.asarray(Wq).T * 0.125, np.float32)
    WkT = np.ascontiguousarray(np.asarray(Wk).T, np.float32)
    WvT = np.ascontiguousarray(np.asarray(Wv).T, np.float32)
    rows_per_core = R // n_cores
    in_maps = []
    for c in range(n_cores):
        r0 = c * rows_per_core
        xT = np.ascontiguousarray(
            hs[r0:r0 + rows_per_core].reshape(rows_per_core * m, dim).T,
            np.float32)
        in_maps.append(dict(
            xT=xT,
            simg=sim[r0:r0 + rows_per_core],
            am=am[r0:r0 + rows_per_core],
            WqT=WqT, WkT=WkT, WvT=WvT,
            bq=np.ascontiguousarray(np.asarray(bq) * 0.125, np.float32),
            bk=np.ascontiguousarray(bk, np.float32),
            bv=np.ascontiguousarray(bv, np.float32),
            ident=ident, selm=selm, bselm=bselm,
        ))
    return in_maps


_CACHE = {}


def _get_compiled(use_bv=True):
    key = ("nc", use_bv)
    if key not in _CACHE:
        nc = bacc.Bacc("TRN2", target_bir_lowering=False, debug=False)
        build_core_kernel(nc, use_bv=use_bv)
        nc.compile()
        _CACHE[key] = nc
    return _CACHE[key]


LAST_EXEC_NS = [None]


def kernel(hidden_states, attention_mask, sim_graph, Wq, bq, Wk, bk, Wv, bv,
           b=4, m=64, seq=256, dim=1024, **_):
    import os
    from concourse.bass_utils import run_bass_kernel_spmd

    use_bv = bool(np.any(np.asarray(bv)))
    nc = _get_compiled(use_bv=use_bv)
    in_maps = _prepare_shards(hidden_states, attention_mask, sim_graph,
                              Wq, bq, Wk, bk, Wv, bv)
    trace = bool(int(os.environ.get("BERT_TRACE", "0")))
    if trace:
        try:  # register the NTFF hook if the middleware didn't
            from antenv.axon_hooks import (get_axon_ntff_profile_hook,
                                           set_axon_ntff_profile_hook)
            if get_axon_ntff_profile_hook() is None:
                from trn_agent_boot.trn_boot import _ntff_profile_via_ctypes
                set_axon_ntff_profile_hook(
                    _ntff_profile_via_ctypes("/opt/axon/libaxon_pjrt.so"))
        except Exception:
            trace = False
    res = run_bass_kernel_spmd(nc, in_maps, list(range(N_CORES)), trace=trace)
    LAST_EXEC_NS[0] = res.exec_time_ns
    R = int(b) * int(seq)
    out = np.concatenate([res.results[c]["out"] for c in range(N_CORES)], axis=0)
    return out.reshape(R, int(m), int(dim))
